# revision 29
# baseline (speedup 1.0000x reference)
"""Trainium2 Bass kernel for capsule routing (nn_Capsule).

Reference computation:
    u_hat = einsum('bic,ce->bie', u_vecs, W).reshape(B, I, N, D).transpose(0,2,1,3)
    b = 0
    for r in range(3):
        c = softmax(b, axis=1)                      # over capsules n
        out = squash(einsum('bni,bnid->bnd', c, u_hat))
        if r < 2: b = einsum('bnd,bnid->bni', out, u_hat)
    return out    # (B, N, D)

Algebraic restructuring (u_hat never materialized; all uses factor through
u_vecs and W):
    round 0:  c uniform = 1/N  ->  out0 = squash((1/N) * W^T (sum_i u[b,i,:]))
    logits[b,i,n] = sum_c u[b,i,c] V[b,c,n],   V[b,c,n] = sum_d W[c,(n,d)] o[b,n,d]
    tt[b,c,n]    = sum_i softmax(logits)[b,i,n] * u[b,i,c]     (T transposed)
    pre[b,n,d]   = sum_c tt[b,c,n] W[c,(n,d)]   -> out = squash(pre)

PE-efficiency notes (LDWEIGHTS cost ~ stationary_columns/1.2 ns, halved by
fast-weight-load which triggers on 128-column non-fp32 stationaries):
  - every routing matmul uses a bf16 128-column stationary (FWL): u chunks for
    tt, u^T chunks for logits, W[:, 128-col] for pre, paired W^T for V,
  - tt is computed directly in [c, n] layout (u chunk stationary, softmax
    weights streamed) so the per-batch T transposes of the naive layout vanish,
  - pre packs capsule pairs into one [c,128] stationary: out quadrants
    [0:64, 0:4] / [64:128, 4:8] hold pre_n / pre_{n+1}; cross-quadrants are
    discarded.  W^T for V is built with the same pairing (one [c,128]
    transpose per capsule pair),
  - squash rsqrt = bit-trick seed + Newton on DVE, keeping the Scalar
    activation table pinned on exp,
  - input DMAs are 6 x 1MB split across the two HW-DGE rings (SP + ACT).

Sharding: data-parallel over batch, 4 batches per core x 8 cores, W replicated.
"""

import numpy as np
from contextlib import ExitStack

import concourse.bass as bass
import concourse.bacc as bacc
import concourse.tile as tile
from concourse import mybir
from concourse.bass_utils import run_bass_kernel_spmd
from concourse.masks import make_identity

B, I, C = 32, 1024, 256
N, D = 32, 64
ND = N * D
ROUTINGS = 3
EPS = 1e-7
NCORES = 8
BL = B // NCORES  # batches per core
IC = I // 128     # i chunks of 128
CK = C // 128     # c chunks of 128
NB = N * BL       # 128 = (n, b) composite
NP = N // 2       # capsule pairs
NDK = ND // 128   # 128-col chunks of the (n,d) axis == NP
F32 = mybir.dt.float32
F32R = mybir.dt.float32r
U32 = mybir.dt.uint32
BF16 = mybir.dt.bfloat16
MULT = mybir.AluOpType.mult
AF = mybir.ActivationFunctionType
RSQRT_MAGIC = 0x5F3759DF


def _r(ap):
    """View an fp32 AP as float32r: single-pass (FP22) PE matmul instead of
    the 2-pass fp32 split walrus emits otherwise."""
    return ap.bitcast(F32R)


def _capsule_body(ctx: ExitStack, tc: tile.TileContext, out_ap, u_ap, w_ap):
    nc = tc.nc

    ctx.enter_context(nc.allow_low_precision(reason="bf16/fp32r matmul path"))

    const = ctx.enter_context(tc.tile_pool(name="const", bufs=1))
    persist = ctx.enter_context(tc.tile_pool(name="persist", bufs=1))
    work = ctx.enter_context(tc.tile_pool(name="work", bufs=4))

    # ---- constants ----
    ident = const.tile([128, 128], F32)
    make_identity(nc, ident[:])
    ident_bf = const.tile([128, 128], BF16)
    make_identity(nc, ident_bf[:])
    ident_r = const.tile([128, 128], F32)
    nc.vector.tensor_copy(out=_r(ident_r[:]), in_=ident[:])
    ones_f = const.tile([128, 2], F32)
    nc.gpsimd.memset(ones_f[:], 1.0)
    ones_bf = const.tile([128, 1], BF16)
    nc.gpsimd.memset(ones_bf[:], 1.0)
    ones_col = const.tile([128, 1], F32)
    nc.vector.tensor_copy(out=_r(ones_col[:]), in_=ones_f[:, 0:1])
    ones_row = const.tile([1, 128], F32)
    nc.vector.tensor_copy(out=_r(ones_row[:]), in_=ones_f[0:1, 0:1].to_broadcast([1, 128]))
    magic = const.tile([1, NB], U32)
    nc.gpsimd.memset(magic[:], RSQRT_MAGIC)

    # ---- persistent SBUF tensors ----
    w_sb = persist.tile([128, CK, ND], F32)       # [c, ck, (n,d)]
    w_bf = persist.tile([128, CK, ND], BF16)      # bf16 copy (pre stationaries)
    wt_ev = persist.tile([64, NP, C], BF16)       # [d, p, c] = W_2p^T (V stationaries)
    wt_od = persist.tile([64, NP, C], BF16)       # [d, p, c] = W_{2p+1}^T
    u_sb = persist.tile([128, BL, IC, C], F32)    # [i, b, ic, c]
    u_bf = persist.tile([128, BL, IC, C], BF16)   # bf16 copy (tt stationaries)
    ut_bf = persist.tile([128, BL, CK, I], BF16)  # [c, b, ck, i] (lg stationaries)
    st_bf = persist.tile([128, CK, BL], BF16)     # [c, ck, b] column sums of u

    # ---- input DMAs: 6 x 1MB, split across the SP and ACT HW-DGE rings ----
    # sync ring: u[b=0], u[b=1]; scalar ring: W, u[b=2], u[b=3]
    def dma_u(eng, b):
        src = bass.AP(
            tensor=u_ap.tensor,
            offset=u_ap.offset + b * I * C,
            ap=[[C, 128], [128 * C, IC], [1, C]],
        )
        eng.dma_start(out=_r(u_sb[:, b, :, :]), in_=_r(src))

    # All DMAs on the SP ring (ACT-issued DMAs wedge the device under this
    # runtime); W interleaved late since its consumers run later
    def dma_w(ck):
        nc.sync.dma_start(
            out=_r(w_sb[:, ck, :]), in_=_r(w_ap[ck * 128:(ck + 1) * 128, :])
        )

    for b in range(BL):
        dma_u(nc.sync, b)
    dma_w(0)
    dma_w(1)

    # ---- setup: casts, transposes, column sums ----
    def _copy(idx, out, in_):
        # PSUM -> SBUF evacuation: only ACT and DVE can read PSUM (and both
        # cast ~3x faster than GpSimd, which is kept off the big-tensor path)
        if idx % 2 == 0:
            nc.scalar.copy(out=out, in_=in_)
        else:
            nc.vector.tensor_copy(out=out, in_=in_)

    with tc.tile_pool(name="ps_su", bufs=3, space="PSUM") as ps_su, \
            tc.tile_pool(name="ps_sw", bufs=2, space="PSUM") as ps_sw, \
            tc.tile_pool(name="ps_st", bufs=1, space="PSUM") as ps_st, \
            nc.named_scope("setup"):
        st_ps = ps_st.tile([128, CK, BL], F32, tag="st")

        def wt_transposes(ck):
            # paired W^T rows from f32 W directly (f32r single-pass, so the
            # PE does not wait on the bf16 W cast); the copy-out casts
            for g in range(NP // 2):
                wt_ps = ps_sw.tile([128, 2, 128], F32, tag="wt")
                for q in range(2):
                    p = 2 * g + q
                    nc.tensor.transpose(
                        _r(wt_ps[:, q, :]),
                        _r(w_sb[:, ck, p * 128:(p + 1) * 128]),
                        _r(ident_r[:]),
                    )
                _copy(
                    ck * 8 + g,
                    wt_ev[:, 2 * g:2 * g + 2, ck * 128:(ck + 1) * 128],
                    wt_ps[0:64, :, :],
                )
                _copy(
                    ck * 8 + g + 1,
                    wt_od[:, 2 * g:2 * g + 2, ck * 128:(ck + 1) * 128],
                    wt_ps[64:128, :, :],
                )

        def u_section(b):
            # cast u[b] to bf16 in halves so transposes chase the DMA
            for h in range(2):
                _copy(
                    h,
                    u_bf[:, b, h * 4:(h + 1) * 4, :],
                    u_sb[:, b, h * 4:(h + 1) * 4, :],
                )
            # transpose 4-chunk groups: ut[c, b, ck, :]
            for ck in range(CK):
                for j in range(IC // 4):
                    ut_ps = ps_su.tile([128, 4, 128], BF16, tag="ut")
                    for t in range(4):
                        icx = 4 * j + t
                        nc.tensor.transpose(
                            ut_ps[:, t, :],
                            u_bf[:, b, icx, ck * 128:(ck + 1) * 128],
                            ident_bf[:],
                        )
                    _copy(
                        b * 4 + ck * 2 + j,
                        ut_bf[:, b, ck, j * 512:(j + 1) * 512],
                        ut_ps[:].rearrange("c t i -> c (t i)"),
                    )
            # column sums st[c, b] = sum_i u[i, c] as ones-matmuls (off the
            # DVE, and not dependent on the transposes' PSUM evacuation)
            for ck in range(CK):
                for ic in range(IC):
                    nc.tensor.matmul(
                        out=st_ps[:, ck, b:b + 1],
                        lhsT=u_bf[:, b, ic, ck * 128:(ck + 1) * 128],
                        rhs=ones_bf[:],
                        start=(ic == 0),
                        stop=(ic == IC - 1),
                    )

        u_section(0)
        u_section(1)
        u_section(2)
        u_section(3)
        # W transposes last: the PE queue never head-of-line blocks on the
        # late-landing W DMA while u work is still available
        wt_transposes(0)
        wt_transposes(1)
        # bf16 W for the pre stationaries: GpSimd (slow but idle) takes the
        # earliest-landing half; ACT/DVE take the rest after their u casts
        nc.gpsimd.tensor_copy(out=w_bf[:, 0, 0:1024], in_=w_sb[:, 0, 0:1024])
        nc.vector.tensor_copy(out=w_bf[:, 0, 1024:2048], in_=w_sb[:, 0, 1024:2048])
        nc.scalar.copy(out=w_bf[:, 1, 0:1024], in_=w_sb[:, 1, 0:1024])
        nc.vector.tensor_copy(out=w_bf[:, 1, 1024:2048], in_=w_sb[:, 1, 1024:2048])
        nc.vector.tensor_copy(out=st_bf[:], in_=st_ps[:])

    ps = ctx.enter_context(tc.tile_pool(name="ps_main", bufs=1, space="PSUM"))
    ps_lg = ctx.enter_context(tc.tile_pool(name="ps_lg", bufs=3, space="PSUM"))
    ps_tt = ctx.enter_context(tc.tile_pool(name="ps_tt", bufs=1, space="PSUM"))
    ps_pre = ctx.enter_context(tc.tile_pool(name="ps_pre", bufs=1, space="PSUM"))

    o_sb = None
    for r in range(ROUTINGS):
        tt_bf = None
        if r > 0:
            # V[b][c, n] = sum_d W[c,(n,d)] o[b,n,d]; stationary = paired W^T
            with nc.named_scope(f"r{r}_v"):
                v_ps = ps.tile([128, CK, N, BL], F32, tag="v")
                for ck in range(CK):
                    for n in range(N):
                        wt = wt_ev if n % 2 == 0 else wt_od
                        nc.tensor.matmul(
                            out=v_ps[:, ck, n, :],
                            lhsT=wt[:, n // 2, ck * 128:(ck + 1) * 128],
                            rhs=o_sb[:, n * BL:(n + 1) * BL],
                            start=True,
                            stop=True,
                        )
                v_bf = work.tile([128, CK, N, BL], BF16, tag="v_bf")
                nc.scalar.copy(out=v_bf[:], in_=v_ps[:])

            # logits[b][i, n] = sum_c ut[c, i] V[c, n]   (all b first: the
            # per-b softmax chains run behind the PE's logits stream)
            lg_tiles = []
            with nc.named_scope(f"r{r}_lg"):
                for b in range(BL):
                    lg_ps = ps_lg.tile([128, IC, N], F32, tag="lg")
                    lg_tiles.append(lg_ps)
                    for ic in range(IC):
                        for ck in range(CK):
                            nc.tensor.matmul(
                                out=lg_ps[:, ic, :],
                                lhsT=ut_bf[:, b, ck, ic * 128:(ic + 1) * 128],
                                rhs=v_bf[:, ck, :, b],
                                start=(ck == 0),
                                stop=(ck == CK - 1),
                            )
            # softmax over n (free dim; logits are O(1), no max-subtraction)
            c_tiles = []
            with nc.named_scope(f"r{r}_sm"):
                for b in range(BL):
                    e_sb = work.tile([128, IC, N], F32, tag="e")
                    nc.scalar.activation(
                        out=e_sb[:], in_=lg_tiles[b][:], func=AF.Exp
                    )
                    s_sb = work.tile([128, IC], F32, tag="s")
                    nc.vector.reduce_sum(
                        out=s_sb[:], in_=e_sb[:], axis=mybir.AxisListType.X
                    )
                    sr_sb = work.tile([128, IC], F32, tag="sr")
                    nc.vector.reciprocal(out=sr_sb[:], in_=s_sb[:])
                    c_bf = work.tile([128, IC, N], BF16, tag="c")
                    c_tiles.append(c_bf)
                    nc.gpsimd.tensor_tensor(
                        c_bf[:],
                        e_sb[:],
                        sr_sb[:, :, None].to_broadcast([128, IC, N]),
                        MULT,
                    )
            # tt[b][c, n] = sum_i u[i, c] c[i, n]   (T transposed, directly)
            with nc.named_scope(f"r{r}_t"):
                tt_bf = work.tile([128, CK, N, BL], BF16, tag="tt")
                for b in range(BL):
                    tt_ps = ps_tt.tile([128, CK, N], F32, tag="tt")
                    for ck in range(CK):
                        for ic in range(IC):
                            nc.tensor.matmul(
                                out=tt_ps[:, ck, :],
                                lhsT=u_bf[:, b, ic, ck * 128:(ck + 1) * 128],
                                rhs=c_tiles[b][:, ic, :],
                                start=(ic == 0),
                                stop=(ic == IC - 1),
                            )
                    _copy(b, tt_bf[:, :, :, b], tt_ps[:])

        # ---------- pre: paired-capsule stationaries, quadrant extraction ----
        # r == 0: rhs = st (uniform c), both row-halves valid for rhs cols 0:4.
        # r >= 1: rhs = [tt_n | tt_{n+1}] (8 cols); quads [0:64,0:4]/[64:128,4:8].
        with nc.named_scope(f"r{r}_pre"):
            fr = BL if r == 0 else 2 * BL
            pre_ps = ps_pre.tile([128, NP, 2 * BL], F32, tag="pre")
            for k in range(NP):
                for ck in range(CK):
                    rhs = (
                        st_bf[:, ck, :] if r == 0
                        else tt_bf[:, ck, 2 * k:2 * k + 2, :]
                    )
                    nc.tensor.matmul(
                        out=pre_ps[:, k, 0:fr],
                        lhsT=w_bf[:, ck, k * 128:(k + 1) * 128],
                        rhs=rhs,
                        start=(ck == 0),
                        stop=(ck == CK - 1),
                    )
            pre_sb = work.tile([64, NB], F32, tag="pre_sb")
            pre_v = pre_sb[:].rearrange("d (k x) -> d k x", x=2 * BL)
            nc.scalar.copy(
                out=pre_v[:, :, 0:BL], in_=pre_ps[0:64, :, 0:BL]
            )
            nc.vector.tensor_copy(
                out=pre_v[:, :, BL:2 * BL],
                in_=pre_ps[64:128, :, (0 if r == 0 else BL):fr],
            )

        # ---------- squash over d (ones-matmul reduction; rsqrt via bit-trick
        # seed + Newton on DVE keeps the ACT table pinned on exp) ----------
        with nc.named_scope(f"r{r}_sq"):
            sq_sb = work.tile([64, NB], F32, tag="sq")
            nc.vector.tensor_mul(_r(sq_sb[:]), pre_sb[:], pre_sb[:])
            ss_ps = ps.tile([1, NB], F32, tag="sqps")
            nc.tensor.matmul(
                out=ss_ps[:], lhsT=_r(ones_col[0:64, :]), rhs=_r(sq_sb[:]),
                start=True, stop=True,
            )
            # x = sum/N^2 + eps for r==0 (squash of pre/N), else sum + eps
            x_sb = work.tile([1, NB], F32, tag="x")
            nc.vector.tensor_scalar(
                out=x_sb[:], in0=ss_ps[:],
                scalar1=(1.0 / (N * N) if r == 0 else 1.0), scalar2=EPS,
                op0=MULT, op1=mybir.AluOpType.add,
            )
            # y0 = bitcast(0x5f3759df - (bitcast(x) >> 1))
            yb_sb = work.tile([1, NB], U32, tag="yb")
            nc.vector.tensor_scalar(
                out=yb_sb[:], in0=x_sb[:].bitcast(U32), scalar1=1, scalar2=None,
                op0=mybir.AluOpType.logical_shift_right,
            )
            y_sb = work.tile([1, NB], F32, tag="y")
            nc.vector.tensor_tensor(
                y_sb[:].bitcast(U32), magic[:], yb_sb[:],
                mybir.AluOpType.subtract,
            )
            # Newton: y <- y * (1.5 - 0.5 x y^2); 1 step mid-routing (the
            # magnitude error only perturbs the next round's logit scale by
            # ~0.2%), 2 steps for the returned round (rsqrt rel err ~4e-6)
            for it in range(1 if r < ROUTINGS - 1 else 2):
                t1 = work.tile([1, NB], F32, tag="nt1")
                nc.vector.tensor_mul(t1[:], y_sb[:], y_sb[:])
                nc.vector.scalar_tensor_tensor(
                    out=t1[:], in0=t1[:], scalar=-0.5, in1=x_sb[:],
                    op0=MULT, op1=MULT,
                )
                y2 = work.tile([1, NB], F32, tag="y")
                nc.vector.scalar_tensor_tensor(
                    out=_r(y2[:]), in0=t1[:], scalar=1.5, in1=y_sb[:],
                    op0=mybir.AluOpType.add, op1=MULT,
                )
                y_sb = y2
            if r == 0:
                nc.vector.tensor_scalar_mul(_r(y_sb[:]), y_sb[:], 1.0 / N)
            rnb_ps = ps.tile([64, NB], F32, tag="rnb")
            nc.tensor.matmul(
                out=rnb_ps[:], lhsT=_r(ones_row[0:1, 0:64]), rhs=_r(y_sb[:]),
                start=True, stop=True,
            )
            if r < ROUTINGS - 1:
                o_sb = work.tile([64, NB], BF16, tag="o_bf")
                nc.vector.tensor_tensor(o_sb[:], pre_sb[:], rnb_ps[:], MULT)
            else:
                o_sb = work.tile([64, NB], F32, tag="o")
                nc.vector.tensor_tensor(_r(o_sb[:]), pre_sb[:], rnb_ps[:], MULT)

    # ---------- write out: out[b, n, d] = o[d, (n,b)] ----------
    with nc.named_scope("out"):
        ot_ps = ps.tile([128, 64], F32, tag="sqps")
        nc.tensor.transpose(ot_ps[:], o_sb[:], ident[0:64, 0:64])
        ot_sb = work.tile([128, 64], F32, tag="ot")
        nc.scalar.copy(out=ot_sb[:], in_=ot_ps[:])
        out_nbd = bass.AP(
            tensor=out_ap.tensor,
            offset=out_ap.offset,
            ap=[[D, N], [N * D, BL], [1, D]],
        )
        nc.sync.dma_start(out=out_nbd, in_=ot_sb[:])


def build_program():
    nc = bacc.Bacc("TRN2", target_bir_lowering=False, debug=False)
    u_ap = nc.dram_tensor("u", [BL, I, C], F32, kind="ExternalInput").ap()
    w_ap = nc.dram_tensor("w", [C, ND], F32, kind="ExternalInput").ap()
    out_ap = nc.dram_tensor("out", [BL, N, D], F32, kind="ExternalOutput").ap()
    with tile.TileContext(nc) as tc:
        with ExitStack() as ctx:
            _capsule_body(ctx, tc, out_ap, u_ap, w_ap)
    nc.compile()
    return nc


_NC = None


def kernel(u_vecs: np.ndarray, W: np.ndarray) -> np.ndarray:
    global _NC
    u = np.ascontiguousarray(np.asarray(u_vecs, dtype=np.float32))
    w = np.ascontiguousarray(np.asarray(W, dtype=np.float32))
    assert u.shape == (B, I, C) and w.shape == (C, ND)
    if _NC is None:
        _NC = build_program()
    in_maps = [
        {"u": u[i * BL:(i + 1) * BL], "w": w} for i in range(NCORES)
    ]
    res = run_bass_kernel_spmd(_NC, in_maps, list(range(NCORES)))
    return np.concatenate(
        [res.results[i]["out"] for i in range(NCORES)], axis=0
    )
